# revision 17
# baseline (speedup 1.0000x reference)
"""Trainium2 Bass kernel for nn_CausalAttention_56873956934253.

Causal attention, B=8, S=1024 (32x32), C=512, 8 heads, D=64, with
weight-normalized QKV projections (PyTorch weight_norm dim=0 style).

Sharding: pure data parallelism over batch — core b handles batch b.
Weights replicated. No collectives.

Per-core dataflow (all shapes hardcoded):
  xq = q[b] as [384, 1024]  (natural layout = X^T, contraction on partitions)
  WT = W^T * diag(g/||v||)  built on-device via fp32 matmuls against a
       diagonal-scale matrix (exact weight-norm folding).
  QT, KT = (WT.T @ X)^T-free: [C, S] layout via PE, bias added on ACT evict.
  Yv     = [S, C] layout (lhsT = X chunks), bias via a K=1 ones-row matmul,
           evicted per-head into a [65]-strided buffer whose 65th columns are 1
           (the ones column makes the AV matmul also produce softmax sums).
  Scores^T[ks, qs] per head = KT_h.T @ QT_h  (K=64; two heads run concurrently
           on disjoint PE row-groups via base partitions 0/64).
           Causal (strictly-lower in [qs,ks] == strictly-upper in [ks,qs]):
           fully-masked tiles skipped, partial tiles restricted to the live
           column range, diagonal 128x128 subblock masked multiplicatively
           post-exp. exp(s/8) fused into the ACT eviction (softmax without max
           subtraction — scores are O(5), fp32 exp is safe).
  AV:      out2aug^T[65, qs] = sum_i Vaug_i.T @ expS_i  (V stationary).
           Row 64 = softmax denominators. reciprocal -> PE-broadcast to
           [64, qs] -> DVE multiply = normalization. qs=0 (fully masked row,
           zeroed by reference's start_mask) handled by zeroing recip col 0.
  Output [C, S] per core -> [8, 512, 32, 32].
"""

import numpy as np
from contextlib import ExitStack

import concourse.bacc as bacc
import concourse.bass as bass
import concourse.tile as tile
import concourse.mybir as mybir
from concourse.bass_utils import run_bass_kernel_spmd
from concourse.masks import make_identity, make_upper_triangular

P = 128
S = 1024
C = 512
KC = 384
NH = 8
D = 64
NB = 8  # batch == cores

F32 = mybir.dt.float32
F32R = mybir.dt.float32r
BF16 = mybir.dt.bfloat16

AF = mybir.ActivationFunctionType
ALU = mybir.AluOpType
AX = mybir.AxisListType


def _r(ap):
    return ap.bitcast(F32R)


def _weight_prep(nc, pools, w_dram, g_dram, b_dram, name, want_bcol):
    """Load natural weight [C, KC], compute per-row weight-norm scale
    g/||v|| as [128, 4] columns, build diag-scale tiles, and emit the
    transposed+scaled weight WT [128(qc%128), 3(qc//128), 512(c)].

    Returns (wt_tile, b_col_or_row)."""
    singles, tmp, ps = pools

    w_nat = singles.tile([P, 4, KC], F32, tag=f"wnat_{name}")
    for g in range(4):
        nc.sync.dma_start(out=w_nat[:, g, :], in_=w_dram.ap()[g * P : (g + 1) * P, :])

    g_col = singles.tile([P, 4], F32, tag=f"gcol_{name}")
    nc.gpsimd.dma_start(out=g_col, in_=g_dram.ap().rearrange("(g p) -> p g", p=P))

    if want_bcol:
        b_sb = singles.tile([P, 4], F32, tag=f"bcol_{name}")
        nc.gpsimd.dma_start(out=b_sb, in_=b_dram.ap().rearrange("(g p) -> p g", p=P))
    else:
        b_sb = singles.tile([1, C], F32R, tag=f"brow_{name}")
        nc.sync.dma_start(out=b_sb, in_=b_dram.ap().rearrange("(one c) -> one c", one=1))

    # scale = g / ||v||  with one Newton refinement of rsqrt for precision
    # (ACT Sqrt table has a loose precision budget).
    sq = tmp.tile([P, 4, KC], F32, tag=f"sq_tmp_{name}")
    ss = tmp.tile([P, 4], F32, tag=f"ss_tmp_{name}")
    for g in range(4):
        nc.vector.tensor_mul(sq[:, g, :], w_nat[:, g, :], w_nat[:, g, :])
        nc.vector.tensor_reduce(ss[:, g : g + 1], sq[:, g, :], axis=AX.X, op=ALU.add)
    r0 = tmp.tile([P, 4], F32, tag=f"r0_tmp_{name}")
    nc.scalar.activation(r0, ss, AF.Sqrt)
    nc.vector.reciprocal(r0, r0)
    h = tmp.tile([P, 4], F32, tag=f"h_tmp_{name}")
    nc.vector.tensor_mul(h, r0, r0)
    nc.vector.tensor_mul(h, h, ss)
    nc.vector.tensor_scalar(out=h, in0=h, scalar1=-0.5, scalar2=1.5, op0=ALU.mult, op1=ALU.add)
    nc.vector.tensor_mul(r0, r0, h)  # r0 = rsqrt(ss), refined
    scale = tmp.tile([P, 4], F32, tag=f"scale_{name}")
    nc.vector.tensor_mul(scale, g_col, r0)

    ident = pools.ident
    ds = singles.tile([P, 4, P], F32, tag=f"ds_{name}")
    for g in range(4):
        nc.vector.tensor_scalar_mul(ds[:, g, :], ident, scale[:, g : g + 1])

    # WT[:, k, 128g:128g+128] = (W[128g:128g+128, 128k:128k+128]).T @ diag(scale_g)
    wt = singles.tile([P, 3, C], F32R, tag=f"wt_{name}")
    for g in range(4):
        for k in range(3):
            pw = ps.tile([P, 512], F32, tag="mm")
            nc.tensor.matmul(
                pw[:, :P],
                lhsT=w_nat[:, g, k * P : (k + 1) * P],
                rhs=ds[:, g, :],
                start=True,
                stop=True,
            )
            nc.scalar.activation(wt[:, k, g * P : (g + 1) * P], pw[:, :P], AF.Copy)
    return wt, b_sb


class _Pools:
    pass


def build_nc():
    nc = bacc.Bacc("TRN2", target_bir_lowering=False, debug=False)

    xq_d = nc.dram_tensor("xq", [KC, S], F32R, kind="ExternalInput")
    xk_d = nc.dram_tensor("xk", [KC, S], F32R, kind="ExternalInput")
    wq_d = nc.dram_tensor("wq", [C, KC], F32, kind="ExternalInput")
    wk_d = nc.dram_tensor("wk", [C, KC], F32, kind="ExternalInput")
    wv_d = nc.dram_tensor("wv", [C, KC], F32, kind="ExternalInput")
    gq_d = nc.dram_tensor("gq", [C], F32, kind="ExternalInput")
    gk_d = nc.dram_tensor("gk", [C], F32, kind="ExternalInput")
    gv_d = nc.dram_tensor("gv", [C], F32, kind="ExternalInput")
    bq_d = nc.dram_tensor("bq", [C], F32, kind="ExternalInput")
    bk_d = nc.dram_tensor("bk", [C], F32, kind="ExternalInput")
    bv_d = nc.dram_tensor("bv", [C], F32R, kind="ExternalInput")
    out_d = nc.dram_tensor("out", [C, S], F32, kind="ExternalOutput")
    rr_d = nc.dram_tensor("rr_scratch", [16, 512], F32)

    with tile.TileContext(nc) as tc:
        with ExitStack() as ctx:
            _body(ctx, tc, xq_d, xk_d,
                  (wq_d, gq_d, bq_d), (wk_d, gk_d, bk_d), (wv_d, gv_d, bv_d),
                  out_d, rr_d)
    nc.compile()
    return nc


def _body(ctx, tc, xq_d, xk_d, wq3, wk3, wv3, out_d, rr_d):
    nc = tc.nc

    singles = ctx.enter_context(tc.tile_pool(name="singles", bufs=1))
    tmp = ctx.enter_context(tc.tile_pool(name="tmp", bufs=2))
    ps = ctx.enter_context(tc.tile_pool(name="ps", bufs=6, space="PSUM"))
    ps_o = ctx.enter_context(tc.tile_pool(name="ps_o", bufs=2, space="PSUM"))
    es_pool = ctx.enter_context(tc.tile_pool(name="es", bufs=27))
    out_pool = ctx.enter_context(tc.tile_pool(name="outp", bufs=4))
    small = ctx.enter_context(tc.tile_pool(name="small", bufs=3))

    pools = _Pools()
    pools.ident = singles.tile([P, P], F32, tag="ident")
    make_identity(nc, pools.ident)

    # strictly-upper 0/1 multiplicative mask: mask[p, f] = 1 iff f > p
    upper01 = singles.tile([P, P], BF16, tag="upper01")
    make_upper_triangular(nc, upper01, val=1.0, diag=False)

    ones_f32 = singles.tile([1, P], F32, tag="ones_f32")
    nc.vector.memset(ones_f32, 1.0)
    ones_row = singles.tile([1, P], F32R, tag="ones_row")
    nc.vector.tensor_copy(ones_row, ones_f32)

    p3 = (singles, tmp, ps)
    wt_q, bq_col = _weight_prep(nc, _attach(p3, pools), *wq3, "q", want_bcol=True)
    wt_k, bk_col = _weight_prep(nc, _attach(p3, pools), *wk3, "k", want_bcol=True)
    wt_v, bv_row = _weight_prep(nc, _attach(p3, pools), *wv3, "v", want_bcol=False)

    # inputs: [384, 1024] -> [128, 3, 1024] (after weight DMAs in queue order)
    xq_s = singles.tile([P, 3, S], F32R, tag="xq_s")
    xk_s = singles.tile([P, 3, S], F32R, tag="xk_s")
    for k in range(3):
        nc.sync.dma_start(out=xq_s[:, k, :], in_=xq_d.ap()[k * P : (k + 1) * P, :])
        nc.sync.dma_start(out=xk_s[:, k, :], in_=xk_d.ap()[k * P : (k + 1) * P, :])

    # ---- Q^T, K^T projections: [128(c%128), 4(c//128), 1024(s)] fp32,
    # emitted per-g inside the attention loop below so projection PE work
    # fills attention latency gaps.
    qt = singles.tile([P, 4, S], F32R, tag="qt")
    kt = singles.tile([P, 4, S], F32R, tag="kt")

    def emit_proj(g):
        for dst, wt, bcol, xs in ((qt, wt_q, bq_col, xq_s), (kt, wt_k, bk_col, xk_s)):
            for j in range(2):
                pp = ps.tile([P, 512], F32, tag="mm")
                for k in range(3):
                    nc.tensor.matmul(
                        pp,
                        lhsT=wt[:, k, g * P : (g + 1) * P],
                        rhs=xs[:, k, j * 512 : (j + 1) * 512],
                        start=(k == 0),
                        stop=(k == 2),
                    )
                nc.scalar.activation(
                    dst[:, g, j * 512 : (j + 1) * 512], pp, AF.Identity,
                    bias=bcol[:, g : g + 1],
                )

    # ---- V projection in [S, C] layout, bf16, heads strided by 65 with a
    # ones column at 65h+64 (pre-set by the big memset below).
    yv = singles.tile([P, 8, NH * 65], BF16, tag="yv")
    nc.gpsimd.memset(yv, 1.0)
    for t in range(8):
        pv = ps.tile([P, 512], F32, tag="mm")
        for k in range(3):
            nc.tensor.matmul(
                pv,
                lhsT=xk_s[:, k, t * P : (t + 1) * P],
                rhs=wt_v[:, k, :],
                start=(k == 0),
                stop=False,
            )
        nc.tensor.matmul(
            pv, lhsT=ones_row, rhs=bv_row, start=False, stop=True,
        )
        for h in range(NH):
            nc.vector.tensor_copy(yv[:, t, 65 * h : 65 * h + 64], pv[:, 64 * h : 64 * h + 64])

    # ---- attention, head pairs share PE via disjoint row groups
    def emit_scores(g4, es_tiles):
        for j in (0, 1):
            for i in range(4 * j + 4):
                r0 = P * max(i - 4 * j, 0)
                for half in (0, 1):
                    pr = slice(64 * half, 64 * half + 64)
                    pst = ps.tile([P, 512], F32, tag="mm")
                    nc.tensor.matmul(
                        pst[:, r0:],
                        lhsT=kt[pr, g4, i * P : (i + 1) * P],
                        rhs=qt[pr, g4, 512 * j + r0 : 512 * (j + 1)],
                        start=True,
                        stop=True,
                    )
                    et = es_pool.tile([P, 512], BF16, tag="es")
                    nc.scalar.activation(et[:, r0:], pst[:, r0:], AF.Exp, scale=0.125)
                    if i - 4 * j >= 0:
                        # mask the diagonal 128x128 subblock (keep f > p)
                        nc.vector.tensor_mul(
                            et[:, r0 : r0 + P], et[:, r0 : r0 + P], upper01
                        )
                    es_tiles[(half, i, j)] = et

    def emit_av(g4, es_tiles):
        for j in (0, 1):
            for half in (0, 1):
                h = 2 * g4 + half
                po = ps_o.tile([65, 512], F32, tag="po")
                n_i = 4 * j + 4
                for i in range(n_i):
                    r0 = P * max(i - 4 * j, 0)
                    nc.tensor.matmul(
                        po[:, r0:],
                        lhsT=yv[:, i, 65 * h : 65 * h + 65],
                        rhs=es_tiles[(half, i, j)][:, r0:],
                        start=(i == 0),
                        stop=(i == n_i - 1),
                    )
                # softmax denominators: guard+recip (DVE), broadcast via DRAM
                # bounce; evict raw numerators immediately to free the PSUM bank
                srow = small.tile([1, 512], F32, tag="srow")
                nc.vector.tensor_scalar(
                    out=srow, in0=po[64:65, :], scalar1=1e-30, scalar2=None, op0=ALU.max
                )
                raw = out_pool.tile([64, 512], F32, tag="raw")
                nc.vector.tensor_copy(raw, po[0:64, :])
                rrow = small.tile([1, 512], F32, tag="rrow")
                nc.vector.reciprocal_approx_fast(rrow, srow)
                slot = h * 2 + j
                nc.sync.dma_start(out=rr_d.ap()[slot : slot + 1, :], in_=rrow)
                bb = small.tile([64, 512], F32, tag="bb")
                rrow_dram = rr_d.ap()[slot : slot + 1, :]
                rrow_bcast = bass.AP(
                    tensor=rrow_dram.tensor,
                    offset=rrow_dram.offset,
                    ap=[[0, 64]] + list(rrow_dram.ap[1:]),
                )
                nc.sync.dma_start(out=bb, in_=rrow_bcast)
                ot = out_pool.tile([64, 512], F32, tag="ot")
                nc.gpsimd.tensor_mul(ot, raw, bb)
                nc.sync.dma_start(
                    out=out_d.ap()[64 * h : 64 * h + 64, 512 * j : 512 * (j + 1)],
                    in_=ot,
                )

    emit_proj(0)
    for g4 in range(4):
        if g4 + 1 < 4:
            emit_proj(g4 + 1)
        es_tiles = {}
        emit_scores(g4, es_tiles)
        emit_av(g4, es_tiles)


def _attach(p3, pools):
    # pools tuple carrying the shared identity tile
    class _T(tuple):
        pass

    t = _T(p3)
    t.ident = pools.ident
    return t


_NC_CACHE = {}


def _get_nc():
    if "nc" not in _NC_CACHE:
        _NC_CACHE["nc"] = build_nc()
    return _NC_CACHE["nc"]


def kernel(q, k, Wq_v, Wq_g, bq, Wk_v, Wk_g, bk, Wv_v, Wv_g, bv, trace=False):
    nc = _get_nc()
    q = np.asarray(q, np.float32)
    k = np.asarray(k, np.float32)
    common = {
        "wq": np.ascontiguousarray(np.asarray(Wq_v, np.float32)),
        "wk": np.ascontiguousarray(np.asarray(Wk_v, np.float32)),
        "wv": np.ascontiguousarray(np.asarray(Wv_v, np.float32)),
        "gq": np.ascontiguousarray(np.asarray(Wq_g, np.float32)),
        "gk": np.ascontiguousarray(np.asarray(Wk_g, np.float32)),
        "gv": np.ascontiguousarray(np.asarray(Wv_g, np.float32)),
        "bq": np.ascontiguousarray(np.asarray(bq, np.float32)),
        "bk": np.ascontiguousarray(np.asarray(bk, np.float32)),
        "bv": np.ascontiguousarray(np.asarray(bv, np.float32)),
    }
    in_maps = []
    for b in range(NB):
        m = dict(common)
        m["xq"] = np.ascontiguousarray(q[b].reshape(KC, S))
        m["xk"] = np.ascontiguousarray(k[b].reshape(KC, S))
        in_maps.append(m)
    res = run_bass_kernel_spmd(nc, in_maps, core_ids=list(range(NB)), trace=trace)
    out = np.stack([res.results[b]["out"] for b in range(NB)])  # [8, 512, 1024]
    out = out.reshape(NB, C, 32, 32).astype(np.float32)
    if trace:
        kernel.last_results = res
    return out
